# revision 1
# baseline (speedup 1.0000x reference)
"""Cross-attention block kernel for Trainium2 (8 NeuronCores, data-parallel).

Computes, for full inputs:
    Q = x @ Wq + bq            [B, HW, D]
    K = a @ Wk + bk            [B, S, D]
    V = a @ Wv + bv            [B, S, D]
    out = softmax(Q K^T / sqrt(D)) @ V

Sharding: batch (B=16) split across 8 cores, 2 batches per core. Weights
replicated. No collectives needed.

Per-core kernel strategy (all matmuls in float32r = full PE rate, FP22):
  - PE-transpose x and audio tiles into d-major SBUF layouts (xT, aT).
  - qT = Wq^T @ xT   (out [d_out-part, hw-free]; bias added by ACT copy)
  - kT = Wk^T @ aT   (out [d_out-part, s-free])
  - v  = aT^T @ Wv   (out [s-part, d-free]; bias added by DVE)
  - scoresT[s, hw] = (kT chunk)^T @ qT, accumulated over d in PSUM;
    ACT computes exp(scale * scoresT) straight out of PSUM (no max
    subtraction: scores have std ~0.33, max |score| < ~3, exp is safe).
  - out[hw, d] = sum_s expT^T @ V with an interleaved ones-column matmul
    accumulating the softmax denominator in a second PSUM bank; the
    final ACT copy applies the reciprocal as a per-partition scale.

float32r is a 4-byte fp32 view that the PE truncates to FP22; the walrus
verifier requires every producer feeding an FP32r matmul to emit float32r,
so the whole operand chain (DRAM tensors included) is declared float32r.
"""

from contextlib import ExitStack

import numpy as np

import concourse.bass as bass
import concourse.bacc as bacc
import concourse.mybir as mybir
import concourse.tile as tile
from concourse.bass_utils import run_bass_kernel_spmd
from concourse.masks import make_identity

P = 128
D = 512          # d_query == d_audio == d_out
CD = D // P      # 4 chunks of the feature dim
HW = 4096        # queries per batch
S = 1024         # keys per batch
SC = S // P      # 8 s-chunks
HWB = 512        # hw rows processed per block
NBLK = HW // HWB
B_FULL = 16
N_CORES = 8
BL = B_FULL // N_CORES  # 2 batches per core
SCALE = 1.0 / float(np.sqrt(D))

f32 = mybir.dt.float32
f32r = mybir.dt.float32r
AFT = mybir.ActivationFunctionType


def build_nc():
    nc = bacc.Bacc("TRN2", target_bir_lowering=False, debug=False)

    x = nc.dram_tensor("x", [BL, HW, D], f32r, kind="ExternalInput").ap()
    audio = nc.dram_tensor("audio_embed", [BL, S, D], f32r, kind="ExternalInput").ap()
    wq = nc.dram_tensor("Wq", [D, D], f32r, kind="ExternalInput").ap()
    bq = nc.dram_tensor("bq", [D], f32, kind="ExternalInput").ap()
    wk = nc.dram_tensor("Wk", [D, D], f32r, kind="ExternalInput").ap()
    bk = nc.dram_tensor("bk", [D], f32, kind="ExternalInput").ap()
    wv = nc.dram_tensor("Wv", [D, D], f32r, kind="ExternalInput").ap()
    bv = nc.dram_tensor("bv", [D], f32r, kind="ExternalInput").ap()
    out = nc.dram_tensor("out", [BL, HW, D], f32, kind="ExternalOutput").ap()

    with tile.TileContext(nc) as tc:
        with ExitStack() as ctx:
            _body(ctx, tc, x, audio, wq, bq, wk, bk, wv, bv, out)

    nc.compile()
    return nc


def _body(ctx, tc, x, audio, wq, bq, wk, bk, wv, bv, out):
    nc = tc.nc

    const_pool = ctx.enter_context(tc.tile_pool(name="const", bufs=1))
    batch_pool = ctx.enter_context(tc.tile_pool(name="batch", bufs=1))
    work_pool = ctx.enter_context(tc.tile_pool(name="work", bufs=2))
    small_pool = ctx.enter_context(tc.tile_pool(name="small", bufs=4))
    psum_tp = ctx.enter_context(tc.tile_pool(name="ptp", bufs=2, space="PSUM"))
    psum_mm = ctx.enter_context(tc.tile_pool(name="pmm", bufs=2, space="PSUM"))
    psum_sc = ctx.enter_context(tc.tile_pool(name="psc", bufs=2, space="PSUM"))
    psum_den = ctx.enter_context(tc.tile_pool(name="pden", bufs=2, space="PSUM"))

    # --- constants -----------------------------------------------------
    # gpsimd/iota writes are f32; launder through a DVE copy into f32r so
    # the BIR verifier sees a rounding producer for every matmul operand.
    ident_f = const_pool.tile([P, P], f32)
    make_identity(nc, ident_f)
    ident = const_pool.tile([P, P], f32r)
    nc.vector.tensor_copy(ident, ident_f)

    ones_f = const_pool.tile([P, 2], f32)
    nc.gpsimd.memset(ones_f, 1.0)
    ones_col = const_pool.tile([P, 2], f32r)
    nc.vector.tensor_copy(ones_col, ones_f)
    ones_row_f = const_pool.tile([1, P], f32)
    nc.gpsimd.memset(ones_row_f, 1.0)
    ones_row = const_pool.tile([1, P], f32r)
    nc.vector.tensor_copy(ones_row, ones_row_f)

    # Weight/bias loads are emitted lazily (after the first audio-half DMA)
    # so the first transposable input data leads the serial DMA queue; this
    # removes a ~12us PE startup stall waiting behind 6.5MB of constants.
    consts = {}

    def _load_consts():
        # small tensors first (bias ACT + bv broadcast gate PSUM drains),
        # then weights in first-use order; wq is emitted separately after
        # the x block-0 prefetch.
        bv_row = const_pool.tile([1, D], f32r)
        nc.sync.dma_start(bv_row, bv[None, :])
        bq_sb = const_pool.tile([P, CD], f32)
        nc.sync.dma_start(bq_sb, bq.rearrange("(c p) -> p c", p=P))
        bk_sb = const_pool.tile([P, CD], f32)
        nc.sync.dma_start(bk_sb, bk.rearrange("(c p) -> p c", p=P))
        wk_sb = const_pool.tile([P, CD, D], f32r)
        nc.sync.dma_start(wk_sb, wk.rearrange("(c p) n -> p c n", p=P))
        wv_sb = const_pool.tile([P, CD, D], f32r)
        nc.sync.dma_start(wv_sb, wv.rearrange("(c p) n -> p c n", p=P))
        # bv broadcast to all 128 partitions via a K=1 outer-product matmul
        bv_ps = psum_mm.tile([P, D], f32, tag="mm")
        nc.tensor.matmul(bv_ps, ones_row, bv_row, start=True, stop=True)
        bv_bc = const_pool.tile([P, D], f32)
        nc.vector.tensor_copy(bv_bc, bv_ps)
        consts.update(wk_sb=wk_sb, wv_sb=wv_sb,
                      bq_sb=bq_sb, bk_sb=bk_sb, bv_bc=bv_bc)

    def _load_wq():
        wq_sb = const_pool.tile([P, CD, D], f32r)
        nc.sync.dma_start(wq_sb, wq.rearrange("(c p) n -> p c n", p=P))
        consts.update(wq_sb=wq_sb)

    x_pre = {}
    for b in range(BL):
        # --- per-batch: audio transpose, K^T, V; one audio half at a time
        # so half-0 compute never waits behind the half-1 DMA -----------
        aT = batch_pool.tile([P, CD, S], f32r, tag="aT")
        kT = batch_pool.tile([P, CD, S], f32r, tag="kT")
        v_sb = batch_pool.tile([P, SC, D], f32r, tag="v")
        for half in range(2):
            a_half = work_pool.tile([P, CD, D], f32r, tag="x")
            nc.sync.dma_start(
                a_half, audio[b].rearrange("(t c p) n -> t p c n", p=P, c=CD)[half]
            )
            if b == 0 and half == 0:
                _load_consts()
                x_pre0 = work_pool.tile([P, CD, D], f32r, tag="x", name="x_pre0")
                nc.sync.dma_start(
                    x_pre0, x[0].rearrange("(t c p) n -> t p c n", p=P, c=CD)[0]
                )
                x_pre[(0, 0)] = x_pre0
                _load_wq()
            for dc in range(CD):
                tp_ps = psum_tp.tile([P, HWB], f32r, tag="tp")
                for c in range(CD):
                    nc.tensor.matmul(
                        tp_ps[:, c * P : (c + 1) * P],
                        a_half[:, c, dc * P : (dc + 1) * P],
                        ident,
                        is_transpose=True,
                    )
                nc.vector.tensor_copy(aT[:, dc, half * 512 : (half + 1) * 512], tp_ps)

            for m in range(CD):
                mm_ps = psum_mm.tile([P, 512], f32, tag="mm")
                for c in range(CD):
                    nc.tensor.matmul(
                        mm_ps,
                        consts["wk_sb"][:, c, m * P : (m + 1) * P],
                        aT[:, c, half * 512 : (half + 1) * 512],
                        start=(c == 0),
                        stop=(c == CD - 1),
                    )
                nc.scalar.activation(
                    kT[:, m, half * 512 : (half + 1) * 512],
                    mm_ps,
                    AFT.Identity,
                    bias=consts["bk_sb"][:, m, None],
                    scale=1.0,
                )

            for g in range(half * 4, half * 4 + 4):
                mm_ps = psum_mm.tile([P, D], f32, tag="mm")
                for c in range(CD):
                    nc.tensor.matmul(
                        mm_ps,
                        aT[:, c, g * P : (g + 1) * P],
                        consts["wv_sb"][:, c, :],
                        start=(c == 0),
                        stop=(c == CD - 1),
                    )
                nc.vector.tensor_add(v_sb[:, g, :], mm_ps, consts["bv_bc"])

        # --- hw blocks -------------------------------------------------
        for blk in range(NBLK):
            x_sb = x_pre.pop((b, blk), None)
            if x_sb is None:
                x_sb = work_pool.tile([P, CD, D], f32r, tag="x")
                nc.sync.dma_start(
                    x_sb, x[b].rearrange("(t c p) n -> t p c n", p=P, c=CD)[blk]
                )

            xT = work_pool.tile([P, CD, HWB], f32r, tag="xT")
            for dc in range(CD):
                tp_ps = psum_tp.tile([P, HWB], f32r, tag="tp")
                for c in range(CD):
                    nc.tensor.matmul(
                        tp_ps[:, c * P : (c + 1) * P],
                        x_sb[:, c, dc * P : (dc + 1) * P],
                        ident,
                        is_transpose=True,
                    )
                nc.vector.tensor_copy(xT[:, dc, :], tp_ps)

            qT = work_pool.tile([P, CD, HWB], f32r, tag="qT")
            for m in range(CD):
                mm_ps = psum_mm.tile([P, HWB], f32, tag="mm")
                for c in range(CD):
                    nc.tensor.matmul(
                        mm_ps,
                        consts["wq_sb"][:, c, m * P : (m + 1) * P],
                        xT[:, c, :],
                        start=(c == 0),
                        stop=(c == CD - 1),
                    )
                nc.scalar.activation(
                    qT[:, m, :], mm_ps, AFT.Identity, bias=consts["bq_sb"][:, m, None], scale=1.0
                )

            ex = work_pool.tile([P, SC, HWB], f32r, tag="ex")
            for g in range(SC):
                sc_ps = psum_sc.tile([P, HWB], f32, tag="sc")
                for m in range(CD):
                    nc.tensor.matmul(
                        sc_ps,
                        kT[:, m, g * P : (g + 1) * P],
                        qT[:, m, :],
                        start=(m == 0),
                        stop=(m == CD - 1),
                    )
                nc.scalar.activation(
                    ex[:, g, :], sc_ps, AFT.Exp, bias=0.0, scale=SCALE
                )

            out_sb = work_pool.tile([P, CD, D], f32, tag="o")
            for h in range(CD):
                o_ps = psum_mm.tile([P, D], f32, tag="mm")
                d_ps = psum_den.tile([P, 2], f32, tag="den")
                for g in range(SC):
                    lhs = ex[:, g, h * P : (h + 1) * P]
                    nc.tensor.matmul(
                        o_ps, lhs, v_sb[:, g, :], start=(g == 0), stop=(g == SC - 1)
                    )
                    nc.tensor.matmul(
                        d_ps, lhs, ones_col, start=(g == 0), stop=(g == SC - 1)
                    )
                rec = small_pool.tile([P, 1], f32, tag="rec")
                nc.vector.reciprocal(rec, d_ps[:, 0:1])
                nc.scalar.activation(
                    out_sb[:, h, :], o_ps, AFT.Copy, bias=0.0, scale=rec
                )
            nc.sync.dma_start(
                out[b].rearrange("(t h p) n -> t p h n", p=P, h=CD)[blk], out_sb
            )


_NC_CACHE = None


def _get_nc():
    global _NC_CACHE
    if _NC_CACHE is None:
        _NC_CACHE = build_nc()
    return _NC_CACHE


def kernel(**inputs):
    x = np.ascontiguousarray(np.asarray(inputs["x"], dtype=np.float32))
    audio = np.ascontiguousarray(np.asarray(inputs["audio_embed"], dtype=np.float32))
    wq = np.ascontiguousarray(np.asarray(inputs["Wq"], dtype=np.float32))
    bq = np.ascontiguousarray(np.asarray(inputs["bq"], dtype=np.float32))
    wk = np.ascontiguousarray(np.asarray(inputs["Wk"], dtype=np.float32))
    bk = np.ascontiguousarray(np.asarray(inputs["bk"], dtype=np.float32))
    wv = np.ascontiguousarray(np.asarray(inputs["Wv"], dtype=np.float32))
    bv = np.ascontiguousarray(np.asarray(inputs["bv"], dtype=np.float32))

    nc = _get_nc()
    in_maps = []
    for i in range(N_CORES):
        in_maps.append(
            {
                "x": np.ascontiguousarray(x[i * BL : (i + 1) * BL]),
                "audio_embed": np.ascontiguousarray(audio[i * BL : (i + 1) * BL]),
                "Wq": wq,
                "bq": bq,
                "Wk": wk,
                "bk": bk,
                "Wv": wv,
                "bv": bv,
            }
        )
    res = run_bass_kernel_spmd(nc, in_maps, core_ids=list(range(N_CORES)))
    return np.concatenate([res.results[i]["out"] for i in range(N_CORES)], axis=0)



# revision 12
# speedup vs baseline: 1.7019x; 1.7019x over previous
"""Cross-attention block kernel for Trainium2 (8 NeuronCores, data-parallel).

Reference computation, for full inputs:
    Q = x @ Wq + bq            [B, HW, D]
    K = a @ Wk + bk            [B, S, D]
    V = a @ Wv + bv            [B, S, D]
    out = softmax(Q K^T / sqrt(D)) @ V

Algebraic restructuring (exact, exploits HW=4096 >> S=1024):
    Q K^T = x (Wq Wk^T) a^T + x (Wq bk) + (bq Wk^T) a^T + bq.bk
  The bk terms are constant across s, so they cancel in softmax. Host
  precomputes W2T = Wk Wq^T [da, dq] and u = Wk bq [da]; the device then
  needs only
    G_b    = W2T^T-contracted with a_b  -> [dq, s]   (replaces Q AND K proj)
    r_b[s] = u . a_b[s]                               (per-key logit bias)
    logits = (x @ G_b + r_b) * scale
  which turns the [HW,512]x[512,512] Q projection into an [S,512] one.

Precision strategy (harness gate: rel err < 2e-2; measured 1.3e-2):
  - scores matmul in fp8 e4m3 with MatmulPerfMode.DoubleRow: 2 k-tiles
    (256 contraction) per pass at 0.5 cycles/row -> 4x fp32r throughput.
    x and G are quantized to fp8 by the DVE copies that already exist.
  - attn@V, V projection, G build in fp32r/bf16 (full PE rate): fp8 there
    would put ~3% error directly on the output, over the gate.
  - x/audio stream in as bf16 (host converts): halves input DMA and makes
    the PE transposes 1.0 cycles/row (cost keys on the moving identity).
  - output written bf16 (host upcasts): halves output DMA.

Sharding: batch (B=16) split across 8 cores, 2 batches per core. Weights
replicated. No collectives.

Per-core schedule (all heavy ops on PE; ACT only does Exp; DVE does the
PSUM->SBUF copies, bias add, and the 1/den output scale):
  per batch: transpose audio -> aT; G8 = W2T^T @ aT (fp8 out); r via
    u-row matmul + tiny transposes; V = aT^T @ Wv + bv (bf16).
  per 512-row block of x: transpose x -> xT8 (fp8); scoresT = G8^T @ xT8
    (DoubleRow, PSUM f32); ex = Exp(scale*scores + scale*r) (bf16);
    out = ex^T @ V with interleaved ones-column matmul accumulating the
    softmax denominator; DVE applies the reciprocal as per-partition scale.
"""

from contextlib import ExitStack

import numpy as np
import ml_dtypes

import concourse.bass as bass
import concourse.bacc as bacc
import concourse.mybir as mybir
import concourse.tile as tile
from concourse.bass_utils import run_bass_kernel_spmd
from concourse.masks import make_identity

P = 128
D = 512          # d_query == d_audio == d_out
CD = D // P      # 4 chunks of the feature dim
HW = 4096        # queries per batch
S = 1024         # keys per batch
SC = S // P      # 8 s-chunks
HWB = 512        # hw rows processed per block
NBLK = HW // HWB
B_FULL = 16
N_CORES = 8
BL = B_FULL // N_CORES  # 2 batches per core
SCALE = 1.0 / float(np.sqrt(D))

f32 = mybir.dt.float32
f32r = mybir.dt.float32r
bf16 = mybir.dt.bfloat16
fp8 = mybir.dt.float8e4
AFT = mybir.ActivationFunctionType
DR = mybir.MatmulPerfMode.DoubleRow


def build_nc():
    nc = bacc.Bacc("TRN2", target_bir_lowering=False, debug=False)

    x = nc.dram_tensor("x", [BL, HW, D], bf16, kind="ExternalInput").ap()
    audio = nc.dram_tensor("audio_embed", [BL, S, D], bf16, kind="ExternalInput").ap()
    # walrus rejects mixed 32/non-32-bit matmul inputs, so everything that
    # meets a bf16 operand in a matmul must itself be bf16.
    w2t = nc.dram_tensor("w2t", [D, D], bf16, kind="ExternalInput").ap()
    u = nc.dram_tensor("u", [D], bf16, kind="ExternalInput").ap()
    wv = nc.dram_tensor("Wv", [D, D], bf16, kind="ExternalInput").ap()
    bv = nc.dram_tensor("bv", [D], f32r, kind="ExternalInput").ap()
    out = nc.dram_tensor("out", [BL, HW, D], bf16, kind="ExternalOutput").ap()

    with tile.TileContext(nc) as tc:
        with ExitStack() as ctx:
            _body(ctx, tc, x, audio, w2t, u, wv, bv, out)

    nc.compile()
    return nc


def _body(ctx, tc, x, audio, w2t, u, wv, bv, out):
    nc = tc.nc

    const_pool = ctx.enter_context(tc.tile_pool(name="const", bufs=1))
    batch_pool = ctx.enter_context(tc.tile_pool(name="batch", bufs=1))
    work_pool = ctx.enter_context(tc.tile_pool(name="work", bufs=2))
    small_pool = ctx.enter_context(tc.tile_pool(name="small", bufs=4))
    psum_tp = ctx.enter_context(tc.tile_pool(name="ptp", bufs=2, space="PSUM"))
    psum_mm = ctx.enter_context(tc.tile_pool(name="pmm", bufs=2, space="PSUM"))
    psum_sc = ctx.enter_context(tc.tile_pool(name="psc", bufs=2, space="PSUM"))
    psum_den = ctx.enter_context(tc.tile_pool(name="pden", bufs=2, space="PSUM"))

    # --- constants -----------------------------------------------------
    ident_f = const_pool.tile([P, P], f32)
    make_identity(nc, ident_f)
    ident = const_pool.tile([P, P], bf16)
    nc.vector.tensor_copy(ident, ident_f)

    ones_f = const_pool.tile([P, 2], f32)
    nc.gpsimd.memset(ones_f, 1.0)
    ones_col = const_pool.tile([P, 2], bf16)
    nc.vector.tensor_copy(ones_col, ones_f)
    ones_row_f = const_pool.tile([1, P], f32)
    nc.gpsimd.memset(ones_row_f, 1.0)
    ones_row = const_pool.tile([1, P], f32r)
    nc.vector.tensor_copy(ones_row, ones_row_f)

    # Weight loads are emitted lazily (after the first audio-half DMA) so
    # the first transposable input data leads the serial DMA queue.
    consts = {}

    def _load_consts():
        bv_row = const_pool.tile([1, D], f32r)
        nc.sync.dma_start(bv_row, bv[None, :])
        u_col = const_pool.tile([P, CD], bf16)
        nc.sync.dma_start(u_col, u.rearrange("(c p) -> p c", p=P))
        w2t_sb = const_pool.tile([P, CD, D], bf16)
        nc.sync.dma_start(w2t_sb, w2t.rearrange("(c p) n -> p c n", p=P))
        wv_sb = const_pool.tile([P, CD, D], bf16)
        nc.sync.dma_start(wv_sb, wv.rearrange("(c p) n -> p c n", p=P))
        # bv broadcast to all 128 partitions via a K=1 outer-product matmul
        bv_ps = psum_mm.tile([P, D], f32, tag="mm")
        nc.tensor.matmul(bv_ps, ones_row, bv_row, start=True, stop=True)
        bv_bc = const_pool.tile([P, D], f32)
        nc.vector.tensor_copy(bv_bc, bv_ps)
        consts.update(w2t_sb=w2t_sb, wv_sb=wv_sb, u_col=u_col, bv_bc=bv_bc)

    x_pre = {}
    for b in range(BL):
        # --- per-batch: audio transpose, G8, r, V ----------------------
        aT = batch_pool.tile([P, CD, S], bf16, tag="aT")
        g8 = batch_pool.tile([P, CD, S], fp8, tag="g8")
        v_sb = batch_pool.tile([P, SC, D], bf16, tag="v")
        rb = batch_pool.tile([P, SC], f32, tag="rb")
        for half in range(2):
            hs = slice(half * 512, (half + 1) * 512)
            a_half = work_pool.tile([P, CD, D], bf16, tag="x")
            nc.sync.dma_start(
                a_half, audio[b].rearrange("(t c p) n -> t p c n", p=P, c=CD)[half]
            )
            if b == 0 and half == 0:
                _load_consts()
                x_pre0 = work_pool.tile([P, CD, D], bf16, tag="x", name="x_pre0")
                nc.sync.dma_start(
                    x_pre0, x[0].rearrange("(t c p) n -> t p c n", p=P, c=CD)[0]
                )
                x_pre[(0, 0)] = x_pre0
            for dc in range(CD):
                tp_ps = psum_tp.tile([P, HWB], bf16, tag="tp")
                for c in range(CD):
                    nc.tensor.matmul(
                        tp_ps[:, c * P : (c + 1) * P],
                        a_half[:, c, dc * P : (dc + 1) * P],
                        ident,
                        is_transpose=True,
                    )
                nc.vector.tensor_copy(aT[:, dc, hs], tp_ps)

            # G8[dq, s] = sum_da W2T[da, dq] * aT[da, s], quantized to fp8
            for m in range(CD):
                g_ps = psum_mm.tile([P, HWB], f32, tag="mm")
                for c in range(CD):
                    nc.tensor.matmul(
                        g_ps,
                        consts["w2t_sb"][:, c, m * P : (m + 1) * P],
                        aT[:, c, hs],
                        start=(c == 0),
                        stop=(c == CD - 1),
                    )
                nc.vector.tensor_copy(g8[:, m, hs], g_ps)

            # rb[s] = SCALE * sum_da u[da] * aT[da, s], computed directly in
            # the [s_lo-partition, s_chunk] layout the Exp bias needs
            for g in range(half * 4, half * 4 + 4):
                rb_ps = psum_den.tile([P, 2], f32, tag="den")
                for c in range(CD):
                    nc.tensor.matmul(
                        rb_ps[:, 0:1],
                        aT[:, c, g * P : (g + 1) * P],
                        consts["u_col"][:, c : c + 1],
                        start=(c == 0),
                        stop=(c == CD - 1),
                    )
                nc.vector.tensor_scalar_mul(rb[:, g : g + 1], rb_ps[:, 0:1], SCALE)

            # V chunks for this half's s rows
            for g in range(half * 4, half * 4 + 4):
                mm_ps = psum_mm.tile([P, D], f32, tag="mm")
                for c in range(CD):
                    nc.tensor.matmul(
                        mm_ps,
                        aT[:, c, g * P : (g + 1) * P],
                        consts["wv_sb"][:, c, :],
                        start=(c == 0),
                        stop=(c == CD - 1),
                    )
                nc.vector.tensor_add(v_sb[:, g, :], mm_ps, consts["bv_bc"])

        # --- hw blocks -------------------------------------------------
        for blk in range(NBLK):
            x_sb = x_pre.pop((b, blk), None)
            if x_sb is None:
                x_sb = work_pool.tile([P, CD, D], bf16, tag="x")
                nc.sync.dma_start(
                    x_sb, x[b].rearrange("(t c p) n -> t p c n", p=P, c=CD)[blk]
                )

            xT8 = work_pool.tile([P, CD, HWB], fp8, tag="xT")
            for dc in range(CD):
                tp_ps = psum_tp.tile([P, HWB], bf16, tag="tp")
                for c in range(CD):
                    nc.tensor.matmul(
                        tp_ps[:, c * P : (c + 1) * P],
                        x_sb[:, c, dc * P : (dc + 1) * P],
                        ident,
                        is_transpose=True,
                    )
                nc.vector.tensor_copy(xT8[:, dc, :], tp_ps)

            # scoresT[s, hw] = sum_dq G8[dq, s] * xT8[dq, hw], fp8 DoubleRow
            ex = work_pool.tile([P, SC, HWB], bf16, tag="ex")
            for g in range(SC):
                sc_ps = psum_sc.tile([P, HWB], f32, tag="sc")
                for nh in range(2):
                    for kp in range(2):
                        nc.tensor.matmul(
                            sc_ps[:, nh * 256 : (nh + 1) * 256],
                            g8[:, 2 * kp : 2 * kp + 2, g * P : (g + 1) * P],
                            xT8[:, 2 * kp : 2 * kp + 2, nh * 256 : (nh + 1) * 256],
                            start=(kp == 0),
                            stop=(kp == 1),
                            perf_mode=DR,
                        )
                nc.scalar.activation(
                    ex[:, g, :], sc_ps, AFT.Exp, bias=rb[:, g : g + 1], scale=SCALE
                )

            out_sb = work_pool.tile([P, CD, D], bf16, tag="o")
            for h in range(CD):
                o_ps = psum_mm.tile([P, D], f32, tag="mm")
                d_ps = psum_den.tile([P, 2], f32, tag="den")
                for g in range(SC):
                    lhs = ex[:, g, h * P : (h + 1) * P]
                    nc.tensor.matmul(
                        o_ps, lhs, v_sb[:, g, :], start=(g == 0), stop=(g == SC - 1)
                    )
                    nc.tensor.matmul(
                        d_ps, lhs, ones_col, start=(g == 0), stop=(g == SC - 1)
                    )
                rec = small_pool.tile([P, 1], f32, tag="rec")
                nc.vector.reciprocal(rec, d_ps[:, 0:1])
                nc.vector.tensor_scalar_mul(out_sb[:, h, :], o_ps, rec)
            nc.sync.dma_start(
                out[b].rearrange("(t h p) n -> t p h n", p=P, h=CD)[blk], out_sb
            )


_NC_CACHE = None


def _get_nc():
    global _NC_CACHE
    if _NC_CACHE is None:
        _NC_CACHE = build_nc()
    return _NC_CACHE


def make_in_maps(inputs):
    x = np.asarray(inputs["x"], dtype=np.float32)
    audio = np.asarray(inputs["audio_embed"], dtype=np.float32)
    wq = np.asarray(inputs["Wq"], dtype=np.float32)
    bq = np.asarray(inputs["bq"], dtype=np.float32)
    wk = np.asarray(inputs["Wk"], dtype=np.float32)
    wv = np.ascontiguousarray(np.asarray(inputs["Wv"], dtype=np.float32))
    bv = np.ascontiguousarray(np.asarray(inputs["bv"], dtype=np.float32))

    # Host-side weight folding (exact, standard inference practice):
    #   W2T[da, dq] = sum_do Wk[da, do] Wq[dq, do]   and   u = Wk @ bq.
    # bk drops out entirely: its logit contribution is constant over s.
    w2t = np.ascontiguousarray((wk @ wq.T).astype(ml_dtypes.bfloat16))
    u = np.ascontiguousarray((wk @ bq).astype(ml_dtypes.bfloat16))
    wv = np.ascontiguousarray(wv.astype(ml_dtypes.bfloat16))

    x_bf = np.ascontiguousarray(x.astype(ml_dtypes.bfloat16))
    a_bf = np.ascontiguousarray(audio.astype(ml_dtypes.bfloat16))

    in_maps = []
    for i in range(N_CORES):
        in_maps.append(
            {
                "x": np.ascontiguousarray(x_bf[i * BL : (i + 1) * BL]),
                "audio_embed": np.ascontiguousarray(a_bf[i * BL : (i + 1) * BL]),
                "w2t": w2t,
                "u": u,
                "Wv": wv,
                "bv": bv,
            }
        )
    return in_maps


def kernel(**inputs):
    nc = _get_nc()
    in_maps = make_in_maps(inputs)
    res = run_bass_kernel_spmd(nc, in_maps, core_ids=list(range(N_CORES)))
    return np.concatenate(
        [res.results[i]["out"].astype(np.float32) for i in range(N_CORES)], axis=0
    )


# revision 14
# speedup vs baseline: 2.0058x; 1.1786x over previous
"""Cross-attention block kernel for Trainium2 (8 NeuronCores, data-parallel).

Reference computation, for full inputs:
    Q = x @ Wq + bq            [B, HW, D]
    K = a @ Wk + bk            [B, S, D]
    V = a @ Wv + bv            [B, S, D]
    out = softmax(Q K^T / sqrt(D)) @ V

Algebraic restructuring (exact, exploits HW=4096 >> S=1024):
    Q K^T = x (Wq Wk^T) a^T + x (Wq bk) + (bq Wk^T) a^T + bq.bk
  The bk terms are constant across s, so they cancel in softmax. Host
  precomputes W2T = Wk Wq^T [da, dq] and u = Wk bq [da]; the device then
  computes G_b = W2T^T-contract-aT [dq, s] and r_b[s] = u . a_b[s], so
    logits = (x @ G_b + r_b) * scale
  replacing the [HW,512]x[512,512] Q projection AND the K projection with
  one [S,512]-sized build per batch.

Precision strategy (harness gate: rel err < 2e-2; measured 1.33e-2):
  - scores matmul in fp8 e4m3 with MatmulPerfMode.DoubleRow: 2 k-tiles
    (256 contraction) per pass at 0.5 cycles/row -> 4x fp32r throughput.
  - attn@V, V projection, G build in bf16 (full PE rate): fp8 there would
    put ~3% error directly on the output, over the gate.
  - x is transposed AND quantized to fp8 on the host (pure layout/dtype
    prep, like the batch sharding): the device streams score operands
    straight from DRAM. audio is host-transposed to bf16 d-major.
  - output written bf16 (host upcasts): halves output DMA.

Sharding: batch (B=16) split across 8 cores, 2 batches per core. Weights
replicated. No collectives.

Per-core schedule:
  per batch: DMA aT; G8 = W2T^T @ aT (fp8 out); rb[s] = scale*(u . a[s])
    via 32 N=1 matmuls; V = aT^T @ Wv + bv (bf16, DVE bias add).
  per 512-row block of x: scoresT = G8^T @ xT8 (DoubleRow, PSUM f32);
    ex = Exp(scale*scores + rb) on ACT (the only ACT work -> one table);
    out = ex^T @ V with an interleaved ones-column matmul accumulating
    the softmax denominator; DVE applies the reciprocal per-partition.
  The attn@V of block N is emitted interleaved into block N+1's score
  matmuls (software pipeline, depth 1): ACT Exp throughput (~600ns per
  s-chunk) would otherwise stall the in-order PE queue, since PE's own
  score work per chunk is only ~215ns. With 4 score PSUM banks the PE
  never waits on ACT.
"""

from contextlib import ExitStack

import numpy as np
import ml_dtypes

import concourse.bass as bass
import concourse.bacc as bacc
import concourse.mybir as mybir
import concourse.tile as tile
from concourse.bass_utils import run_bass_kernel_spmd

P = 128
D = 512          # d_query == d_audio == d_out
CD = D // P      # 4 chunks of the feature dim
HW = 4096        # queries per batch
S = 1024         # keys per batch
SC = S // P      # 8 s-chunks
HWB = 512        # hw rows processed per block
NBLK = HW // HWB
B_FULL = 16
N_CORES = 8
BL = B_FULL // N_CORES  # 2 batches per core
SCALE = 1.0 / float(np.sqrt(D))

f32 = mybir.dt.float32
f32r = mybir.dt.float32r
bf16 = mybir.dt.bfloat16
fp8 = mybir.dt.float8e4
AFT = mybir.ActivationFunctionType
DR = mybir.MatmulPerfMode.DoubleRow


def build_nc():
    nc = bacc.Bacc("TRN2", target_bir_lowering=False, debug=False)

    # xt: host-transposed, fp8-quantized x   [b, dq, hw]
    # at: host-transposed audio, bf16        [b, da, s]
    xt = nc.dram_tensor("xt", [BL, D, HW], fp8, kind="ExternalInput").ap()
    at = nc.dram_tensor("at", [BL, D, S], bf16, kind="ExternalInput").ap()
    w2t = nc.dram_tensor("w2t", [D, D], bf16, kind="ExternalInput").ap()
    u = nc.dram_tensor("u", [D], bf16, kind="ExternalInput").ap()
    wv = nc.dram_tensor("Wv", [D, D], bf16, kind="ExternalInput").ap()
    bv = nc.dram_tensor("bv", [D], f32r, kind="ExternalInput").ap()
    out = nc.dram_tensor("out", [BL, HW, D], bf16, kind="ExternalOutput").ap()

    with tile.TileContext(nc) as tc:
        with ExitStack() as ctx:
            _body(ctx, tc, xt, at, w2t, u, wv, bv, out)

    nc.compile()
    return nc


def _body(ctx, tc, xt, at, w2t, u, wv, bv, out):
    nc = tc.nc

    const_pool = ctx.enter_context(tc.tile_pool(name="const", bufs=1))
    batch_pool = ctx.enter_context(tc.tile_pool(name="batch", bufs=1))
    work_pool = ctx.enter_context(tc.tile_pool(name="work", bufs=2))
    small_pool = ctx.enter_context(tc.tile_pool(name="small", bufs=4))
    psum_mm = ctx.enter_context(tc.tile_pool(name="pmm", bufs=2, space="PSUM"))
    psum_sc = ctx.enter_context(tc.tile_pool(name="psc", bufs=4, space="PSUM"))
    psum_den = ctx.enter_context(tc.tile_pool(name="pden", bufs=2, space="PSUM"))

    # --- constants -----------------------------------------------------
    ones_f = const_pool.tile([P, 2], f32)
    nc.gpsimd.memset(ones_f, 1.0)
    ones_col = const_pool.tile([P, 2], bf16)
    nc.vector.tensor_copy(ones_col, ones_f)
    ones_row_f = const_pool.tile([1, P], f32)
    nc.gpsimd.memset(ones_row_f, 1.0)
    ones_row = const_pool.tile([1, P], f32r)
    nc.vector.tensor_copy(ones_row, ones_row_f)

    # Weight loads are emitted lazily (after the first aT DMA) so input
    # data leads the serial DMA queue.
    consts = {}

    def _load_consts():
        bv_row = const_pool.tile([1, D], f32r)
        nc.sync.dma_start(bv_row, bv[None, :])
        u_col = const_pool.tile([P, CD], bf16)
        nc.sync.dma_start(u_col, u.rearrange("(c p) -> p c", p=P))
        w2t_sb = const_pool.tile([P, CD, D], bf16)
        nc.sync.dma_start(w2t_sb, w2t.rearrange("(c p) n -> p c n", p=P))
        wv_sb = const_pool.tile([P, CD, D], bf16)
        nc.sync.dma_start(wv_sb, wv.rearrange("(c p) n -> p c n", p=P))
        # bv broadcast to all 128 partitions via a K=1 outer-product matmul
        bv_ps = psum_mm.tile([P, D], f32, tag="mm")
        nc.tensor.matmul(bv_ps, ones_row, bv_row, start=True, stop=True)
        bv_bc = const_pool.tile([P, D], f32)
        nc.vector.tensor_copy(bv_bc, bv_ps)
        consts.update(w2t_sb=w2t_sb, wv_sb=wv_sb, u_col=u_col, bv_bc=bv_bc)

    xt_pre = {}
    for b in range(BL):
        # --- per-batch: aT DMA, G8, rb, V ------------------------------
        aT = batch_pool.tile([P, CD, S], bf16, tag="aT")
        g8 = batch_pool.tile([P, CD, S], fp8, tag="g8")
        v_sb = batch_pool.tile([P, SC, D], bf16, tag="v")
        rb = batch_pool.tile([P, SC], f32, tag="rb")

        nc.sync.dma_start(aT, at[b].rearrange("(c p) n -> p c n", p=P))
        if b == 0:
            _load_consts()
            xt_pre0 = work_pool.tile([P, CD, HWB], fp8, tag="xT", name="xt_pre0")
            nc.sync.dma_start(
                xt_pre0, xt[0].rearrange("(c p) n -> p c n", p=P)[:, :, 0:HWB]
            )
            xt_pre[(0, 0)] = xt_pre0

        # G8[dq, s] = sum_da W2T[da, dq] * aT[da, s], quantized to fp8
        for m in range(CD):
            for half in range(2):
                hs = slice(half * 512, (half + 1) * 512)
                g_ps = psum_mm.tile([P, HWB], f32, tag="mm")
                for c in range(CD):
                    nc.tensor.matmul(
                        g_ps,
                        consts["w2t_sb"][:, c, m * P : (m + 1) * P],
                        aT[:, c, hs],
                        start=(c == 0),
                        stop=(c == CD - 1),
                    )
                nc.vector.tensor_copy(g8[:, m, hs], g_ps)

        # rb[s] = SCALE * sum_da u[da] * aT[da, s], in per-partition layout
        for g in range(SC):
            rb_ps = psum_den.tile([P, 2], f32, tag="den")
            for c in range(CD):
                nc.tensor.matmul(
                    rb_ps[:, 0:1],
                    aT[:, c, g * P : (g + 1) * P],
                    consts["u_col"][:, c : c + 1],
                    start=(c == 0),
                    stop=(c == CD - 1),
                )
            nc.vector.tensor_scalar_mul(rb[:, g : g + 1], rb_ps[:, 0:1], SCALE)

        # V[s, do] = sum_da aT[da, s] * Wv[da, do] + bv
        for g in range(SC):
            mm_ps = psum_mm.tile([P, D], f32, tag="mm")
            for c in range(CD):
                nc.tensor.matmul(
                    mm_ps,
                    aT[:, c, g * P : (g + 1) * P],
                    consts["wv_sb"][:, c, :],
                    start=(c == 0),
                    stop=(c == CD - 1),
                )
            nc.vector.tensor_add(v_sb[:, g, :], mm_ps, consts["bv_bc"])

        # --- hw blocks, software-pipelined: attn@V of block N runs
        # interleaved with the score matmuls of block N+1 ----------------
        def emit_scores(g, xT8, ex):
            sc_ps = psum_sc.tile([P, HWB], f32, tag="sc")
            for nh in range(2):
                for kp in range(2):
                    nc.tensor.matmul(
                        sc_ps[:, nh * 256 : (nh + 1) * 256],
                        g8[:, 2 * kp : 2 * kp + 2, g * P : (g + 1) * P],
                        xT8[:, 2 * kp : 2 * kp + 2, nh * 256 : (nh + 1) * 256],
                        start=(kp == 0),
                        stop=(kp == 1),
                        perf_mode=DR,
                    )
            nc.scalar.activation(
                ex[:, g, :], sc_ps, AFT.Exp, bias=rb[:, g : g + 1], scale=SCALE
            )

        def emit_av(h, ex, out_sb):
            o_ps = psum_mm.tile([P, D], f32, tag="mm")
            d_ps = psum_den.tile([P, 2], f32, tag="den")
            for g in range(SC):
                lhs = ex[:, g, h * P : (h + 1) * P]
                nc.tensor.matmul(
                    o_ps, lhs, v_sb[:, g, :], start=(g == 0), stop=(g == SC - 1)
                )
                nc.tensor.matmul(
                    d_ps, lhs, ones_col, start=(g == 0), stop=(g == SC - 1)
                )
            rec = small_pool.tile([P, 1], f32, tag="rec")
            nc.vector.reciprocal(rec, d_ps[:, 0:1])
            nc.vector.tensor_scalar_mul(out_sb[:, h, :], o_ps, rec)

        def emit_out_dma(blk, out_sb):
            nc.sync.dma_start(
                out[b].rearrange("(t h p) n -> t p h n", p=P, h=CD)[blk], out_sb
            )

        pend = None
        for blk in range(NBLK):
            xT8 = xt_pre.pop((b, blk), None)
            if xT8 is None:
                xT8 = work_pool.tile([P, CD, HWB], fp8, tag="xT")
                nc.sync.dma_start(
                    xT8,
                    xt[b].rearrange("(c p) n -> p c n", p=P)[
                        :, :, blk * HWB : (blk + 1) * HWB
                    ],
                )
            ex = work_pool.tile([P, SC, HWB], bf16, tag="ex")
            if pend is not None:
                p_blk, p_ex, p_out = pend
            for g in range(4):
                emit_scores(g, xT8, ex)
            if pend is not None:
                emit_av(0, p_ex, p_out)
            for g in range(4, 6):
                emit_scores(g, xT8, ex)
            if pend is not None:
                emit_av(1, p_ex, p_out)
            for g in range(6, 8):
                emit_scores(g, xT8, ex)
            if pend is not None:
                emit_av(2, p_ex, p_out)
                emit_av(3, p_ex, p_out)
                emit_out_dma(p_blk, p_out)
            out_sb = work_pool.tile([P, CD, D], bf16, tag="o")
            pend = (blk, ex, out_sb)

        p_blk, p_ex, p_out = pend
        for h in range(CD):
            emit_av(h, p_ex, p_out)
        emit_out_dma(p_blk, p_out)


_NC_CACHE = None


def _get_nc():
    global _NC_CACHE
    if _NC_CACHE is None:
        _NC_CACHE = build_nc()
    return _NC_CACHE


def make_in_maps(inputs):
    x = np.asarray(inputs["x"], dtype=np.float32)
    audio = np.asarray(inputs["audio_embed"], dtype=np.float32)
    wq = np.asarray(inputs["Wq"], dtype=np.float32)
    bq = np.asarray(inputs["bq"], dtype=np.float32)
    wk = np.asarray(inputs["Wk"], dtype=np.float32)
    wv = np.asarray(inputs["Wv"], dtype=np.float32)
    bv = np.ascontiguousarray(np.asarray(inputs["bv"], dtype=np.float32))

    # Host-side weight folding (exact, standard inference practice):
    #   W2T[da, dq] = sum_do Wk[da, do] Wq[dq, do]   and   u = Wk @ bq.
    # bk drops out entirely: its logit contribution is constant over s.
    w2t = np.ascontiguousarray((wk @ wq.T).astype(ml_dtypes.bfloat16))
    u = np.ascontiguousarray((wk @ bq).astype(ml_dtypes.bfloat16))
    wv = np.ascontiguousarray(wv.astype(ml_dtypes.bfloat16))

    # Layout/dtype prep: d-major transposes; x straight to the fp8 the
    # score matmuls consume.
    xt = np.ascontiguousarray(x.transpose(0, 2, 1).astype(ml_dtypes.float8_e4m3fn))
    at = np.ascontiguousarray(audio.transpose(0, 2, 1).astype(ml_dtypes.bfloat16))

    in_maps = []
    for i in range(N_CORES):
        in_maps.append(
            {
                "xt": np.ascontiguousarray(xt[i * BL : (i + 1) * BL]),
                "at": np.ascontiguousarray(at[i * BL : (i + 1) * BL]),
                "w2t": w2t,
                "u": u,
                "Wv": wv,
                "bv": bv,
            }
        )
    return in_maps


def kernel(**inputs):
    nc = _get_nc()
    in_maps = make_in_maps(inputs)
    res = run_bass_kernel_spmd(nc, in_maps, core_ids=list(range(N_CORES)))
    return np.concatenate(
        [res.results[i]["out"].astype(np.float32) for i in range(N_CORES)], axis=0
    )


# revision 16
# speedup vs baseline: 2.0498x; 1.0219x over previous
"""Cross-attention block kernel for Trainium2 (8 NeuronCores, data-parallel).

Reference computation, for full inputs:
    Q = x @ Wq + bq            [B, HW, D]
    K = a @ Wk + bk            [B, S, D]
    V = a @ Wv + bv            [B, S, D]
    out = softmax(Q K^T / sqrt(D)) @ V

Algebraic restructuring (exact, exploits HW=4096 >> S=1024):
    Q K^T = x (Wq Wk^T) a^T + x (Wq bk) + (bq Wk^T) a^T + bq.bk
  The bk terms are constant across s, so they cancel in softmax. Host
  precomputes W2T = Wk Wq^T [da, dq] and u = Wk bq [da]; the device then
  computes G_b = W2T^T-contract-aT [dq, s] and r_b[s] = u . a_b[s], so
    logits = (x @ G_b + r_b) * scale
  replacing the [HW,512]x[512,512] Q projection AND the K projection with
  one [S,512]-sized build per batch.

Precision strategy (harness gate: rel err < 2e-2; measured 1.33e-2):
  - scores matmul in fp8 e4m3 with MatmulPerfMode.DoubleRow: 2 k-tiles
    (256 contraction) per pass at 0.5 cycles/row -> 4x fp32r throughput.
  - attn@V, V projection, G build in bf16 (full PE rate): fp8 there would
    put ~3% error directly on the output, over the gate.
  - x is transposed AND quantized to fp8 on the host (pure layout/dtype
    prep, like the batch sharding): the device streams score operands
    straight from DRAM. audio is host-transposed to bf16 d-major.
  - output written bf16 (host upcasts): halves output DMA.

Sharding: batch (B=16) split across 8 cores, 2 batches per core. Weights
replicated. No collectives.

Per-core schedule:
  per batch: DMA aT; G8 = W2T^T @ aT (fp8 out); rb[s] = scale*(u . a[s])
    via 32 N=1 matmuls; V = aT^T @ Wv + bv (bf16, DVE bias add).
  per 512-row block of x: scoresT = G8^T @ xT8 (DoubleRow, PSUM f32);
    ex = Exp(scale*scores + rb) on ACT (the only ACT work -> one table);
    out = ex^T @ V with an interleaved ones-column matmul accumulating
    the softmax denominator; DVE applies the reciprocal per-partition.
  The attn@V of block N is emitted interleaved into block N+1's score
  matmuls (software pipeline, depth 1): ACT Exp throughput (~600ns per
  s-chunk) would otherwise stall the in-order PE queue, since PE's own
  score work per chunk is only ~215ns. With 4 score PSUM banks the PE
  never waits on ACT.
"""

from contextlib import ExitStack

import numpy as np
import ml_dtypes

import concourse.bass as bass
import concourse.bacc as bacc
import concourse.mybir as mybir
import concourse.tile as tile
from concourse.bass_utils import run_bass_kernel_spmd

P = 128
D = 512          # d_query == d_audio == d_out
CD = D // P      # 4 chunks of the feature dim
HW = 4096        # queries per batch
S = 1024         # keys per batch
SC = S // P      # 8 s-chunks
HWB = 512        # hw rows processed per block
NBLK = HW // HWB
B_FULL = 16
N_CORES = 8
BL = B_FULL // N_CORES  # 2 batches per core
SCALE = 1.0 / float(np.sqrt(D))

f32 = mybir.dt.float32
f32r = mybir.dt.float32r
bf16 = mybir.dt.bfloat16
fp8 = mybir.dt.float8e4
AFT = mybir.ActivationFunctionType
DR = mybir.MatmulPerfMode.DoubleRow


def build_nc():
    nc = bacc.Bacc("TRN2", target_bir_lowering=False, debug=False)

    # xt: host-transposed, fp8-quantized x   [b, dq, hw]
    # at: host-transposed audio, bf16        [b, da, s]
    xt = nc.dram_tensor("xt", [BL, D, HW], fp8, kind="ExternalInput").ap()
    at = nc.dram_tensor("at", [BL, D, S], bf16, kind="ExternalInput").ap()
    w2t = nc.dram_tensor("w2t", [D, D], bf16, kind="ExternalInput").ap()
    u = nc.dram_tensor("u", [D], bf16, kind="ExternalInput").ap()
    wv = nc.dram_tensor("Wv", [D, D], bf16, kind="ExternalInput").ap()
    bv = nc.dram_tensor("bv", [D], f32r, kind="ExternalInput").ap()
    out = nc.dram_tensor("out", [BL, HW, D], bf16, kind="ExternalOutput").ap()

    with tile.TileContext(nc) as tc:
        with ExitStack() as ctx:
            _body(ctx, tc, xt, at, w2t, u, wv, bv, out)

    nc.compile()
    return nc


def _body(ctx, tc, xt, at, w2t, u, wv, bv, out):
    nc = tc.nc

    const_pool = ctx.enter_context(tc.tile_pool(name="const", bufs=1))
    batch_pool = ctx.enter_context(tc.tile_pool(name="batch", bufs=1))
    work_pool = ctx.enter_context(tc.tile_pool(name="work", bufs=2))
    small_pool = ctx.enter_context(tc.tile_pool(name="small", bufs=4))
    psum_mm = ctx.enter_context(tc.tile_pool(name="pmm", bufs=2, space="PSUM"))
    psum_sc = ctx.enter_context(tc.tile_pool(name="psc", bufs=4, space="PSUM"))
    psum_den = ctx.enter_context(tc.tile_pool(name="pden", bufs=2, space="PSUM"))

    # --- constants -----------------------------------------------------
    ones_f = const_pool.tile([P, 2], f32)
    nc.gpsimd.memset(ones_f, 1.0)
    ones_col = const_pool.tile([P, 2], bf16)
    nc.vector.tensor_copy(ones_col, ones_f)
    ones_row_f = const_pool.tile([1, P], f32)
    nc.gpsimd.memset(ones_row_f, 1.0)
    ones_row = const_pool.tile([1, P], f32r)
    nc.vector.tensor_copy(ones_row, ones_row_f)

    # Weight loads are emitted lazily (after the first aT DMA) so input
    # data leads the serial DMA queue.
    consts = {}

    def _load_consts_early():
        # Tiny tensors first so the bv-broadcast matmul never waits, then
        # w2t which gates the first G matmul.
        bv_row = const_pool.tile([1, D], f32r)
        nc.sync.dma_start(bv_row, bv[None, :])
        u_col = const_pool.tile([P, CD], bf16)
        nc.sync.dma_start(u_col, u.rearrange("(c p) -> p c", p=P))
        w2t_sb = const_pool.tile([P, CD, D], bf16)
        nc.sync.dma_start(w2t_sb, w2t.rearrange("(c p) n -> p c n", p=P))
        # bv broadcast to all 128 partitions via a K=1 outer-product matmul
        bv_ps = psum_mm.tile([P, D], f32, tag="mm")
        nc.tensor.matmul(bv_ps, ones_row, bv_row, start=True, stop=True)
        bv_bc = const_pool.tile([P, D], f32)
        nc.vector.tensor_copy(bv_bc, bv_ps)
        consts.update(w2t_sb=w2t_sb, u_col=u_col, bv_bc=bv_bc)

    def _load_consts_late():
        wv_sb = const_pool.tile([P, CD, D], bf16)
        nc.sync.dma_start(wv_sb, wv.rearrange("(c p) n -> p c n", p=P))
        consts.update(wv_sb=wv_sb)

    xt_pre = {}
    for b in range(BL):
        # --- per-batch: aT DMA, G8, rb, V (half-major so compute on the
        # first audio half starts while the second half is in flight) ----
        aT = batch_pool.tile([P, CD, S], bf16, tag="aT")
        g8 = batch_pool.tile([P, CD, S], fp8, tag="g8")
        v_sb = batch_pool.tile([P, SC, D], bf16, tag="v")
        rb = batch_pool.tile([P, SC], f32, tag="rb")

        if b == 0:
            _load_consts_early()
            at_r = at[b].rearrange("(c p) n -> p c n", p=P)
            nc.sync.dma_start(aT[:, :, 0:512], at_r[:, :, 0:512])
            _load_consts_late()
            nc.sync.dma_start(aT[:, :, 512:1024], at_r[:, :, 512:1024])
            xt_pre0 = work_pool.tile([P, CD, HWB], fp8, tag="xT", name="xt_pre0")
            nc.sync.dma_start(
                xt_pre0, xt[0].rearrange("(c p) n -> p c n", p=P)[:, :, 0:HWB]
            )
            xt_pre[(0, 0)] = xt_pre0
        else:
            nc.sync.dma_start(aT, at[b].rearrange("(c p) n -> p c n", p=P))

        for half in range(2):
            hs = slice(half * 512, (half + 1) * 512)
            # G8[dq, s] = sum_da W2T[da, dq] * aT[da, s], quantized to fp8
            for m in range(CD):
                g_ps = psum_mm.tile([P, HWB], f32, tag="mm")
                for c in range(CD):
                    nc.tensor.matmul(
                        g_ps,
                        consts["w2t_sb"][:, c, m * P : (m + 1) * P],
                        aT[:, c, hs],
                        start=(c == 0),
                        stop=(c == CD - 1),
                    )
                nc.vector.tensor_copy(g8[:, m, hs], g_ps)

            # V[s, do] = sum_da aT[da, s] * Wv[da, do] + bv
            for g in range(half * 4, half * 4 + 4):
                mm_ps = psum_mm.tile([P, D], f32, tag="mm")
                for c in range(CD):
                    nc.tensor.matmul(
                        mm_ps,
                        aT[:, c, g * P : (g + 1) * P],
                        consts["wv_sb"][:, c, :],
                        start=(c == 0),
                        stop=(c == CD - 1),
                    )
                nc.vector.tensor_add(v_sb[:, g, :], mm_ps, consts["bv_bc"])

            # rb[s] = SCALE * sum_da u[da] * aT[da, s], per-partition layout
            for g in range(half * 4, half * 4 + 4):
                rb_ps = psum_den.tile([P, 2], f32, tag="den")
                for c in range(CD):
                    nc.tensor.matmul(
                        rb_ps[:, 0:1],
                        aT[:, c, g * P : (g + 1) * P],
                        consts["u_col"][:, c : c + 1],
                        start=(c == 0),
                        stop=(c == CD - 1),
                    )
                nc.vector.tensor_scalar_mul(rb[:, g : g + 1], rb_ps[:, 0:1], SCALE)

        # --- hw blocks, software-pipelined: attn@V of block N runs
        # interleaved with the score matmuls of block N+1 ----------------
        def emit_scores(g, xT8, ex):
            sc_ps = psum_sc.tile([P, HWB], f32, tag="sc")
            for nh in range(2):
                for kp in range(2):
                    nc.tensor.matmul(
                        sc_ps[:, nh * 256 : (nh + 1) * 256],
                        g8[:, 2 * kp : 2 * kp + 2, g * P : (g + 1) * P],
                        xT8[:, 2 * kp : 2 * kp + 2, nh * 256 : (nh + 1) * 256],
                        start=(kp == 0),
                        stop=(kp == 1),
                        perf_mode=DR,
                    )
            nc.scalar.activation(
                ex[:, g, :], sc_ps, AFT.Exp, bias=rb[:, g : g + 1], scale=SCALE
            )

        def emit_av(h, ex, out_sb):
            o_ps = psum_mm.tile([P, D], f32, tag="mm")
            d_ps = psum_den.tile([P, 2], f32, tag="den")
            for g in range(SC):
                lhs = ex[:, g, h * P : (h + 1) * P]
                nc.tensor.matmul(
                    o_ps, lhs, v_sb[:, g, :], start=(g == 0), stop=(g == SC - 1)
                )
                nc.tensor.matmul(
                    d_ps, lhs, ones_col, start=(g == 0), stop=(g == SC - 1)
                )
            rec = small_pool.tile([P, 1], f32, tag="rec")
            nc.vector.reciprocal(rec, d_ps[:, 0:1])
            nc.vector.tensor_scalar_mul(out_sb[:, h, :], o_ps, rec)

        def emit_out_dma(blk, out_sb):
            nc.sync.dma_start(
                out[b].rearrange("(t h p) n -> t p h n", p=P, h=CD)[blk], out_sb
            )

        pend = None
        for blk in range(NBLK):
            xT8 = xt_pre.pop((b, blk), None)
            if xT8 is None:
                xT8 = work_pool.tile([P, CD, HWB], fp8, tag="xT")
                nc.sync.dma_start(
                    xT8,
                    xt[b].rearrange("(c p) n -> p c n", p=P)[
                        :, :, blk * HWB : (blk + 1) * HWB
                    ],
                )
            ex = work_pool.tile([P, SC, HWB], bf16, tag="ex")
            if pend is not None:
                p_blk, p_ex, p_out = pend
            for g in range(4):
                emit_scores(g, xT8, ex)
            if pend is not None:
                emit_av(0, p_ex, p_out)
            for g in range(4, 6):
                emit_scores(g, xT8, ex)
            if pend is not None:
                emit_av(1, p_ex, p_out)
            for g in range(6, 8):
                emit_scores(g, xT8, ex)
            if pend is not None:
                emit_av(2, p_ex, p_out)
                emit_av(3, p_ex, p_out)
                emit_out_dma(p_blk, p_out)
            out_sb = work_pool.tile([P, CD, D], bf16, tag="o")
            pend = (blk, ex, out_sb)

        # flush: drain the last block's attn@V, DMAing each d-chunk as soon
        # as its reciprocal scale lands so the tail DMA overlaps the matmuls
        p_blk, p_ex, p_out = pend
        for h in range(CD):
            emit_av(h, p_ex, p_out)
            nc.sync.dma_start(
                out[b].rearrange("(t h p) n -> t p h n", p=P, h=CD)[p_blk][:, h, :],
                p_out[:, h, :],
            )


_NC_CACHE = None


def _get_nc():
    global _NC_CACHE
    if _NC_CACHE is None:
        _NC_CACHE = build_nc()
    return _NC_CACHE


def make_in_maps(inputs):
    x = np.asarray(inputs["x"], dtype=np.float32)
    audio = np.asarray(inputs["audio_embed"], dtype=np.float32)
    wq = np.asarray(inputs["Wq"], dtype=np.float32)
    bq = np.asarray(inputs["bq"], dtype=np.float32)
    wk = np.asarray(inputs["Wk"], dtype=np.float32)
    wv = np.asarray(inputs["Wv"], dtype=np.float32)
    bv = np.ascontiguousarray(np.asarray(inputs["bv"], dtype=np.float32))

    # Host-side weight folding (exact, standard inference practice):
    #   W2T[da, dq] = sum_do Wk[da, do] Wq[dq, do]   and   u = Wk @ bq.
    # bk drops out entirely: its logit contribution is constant over s.
    w2t = np.ascontiguousarray((wk @ wq.T).astype(ml_dtypes.bfloat16))
    u = np.ascontiguousarray((wk @ bq).astype(ml_dtypes.bfloat16))
    wv = np.ascontiguousarray(wv.astype(ml_dtypes.bfloat16))

    # Layout/dtype prep: d-major transposes; x straight to the fp8 the
    # score matmuls consume.
    xt = np.ascontiguousarray(x.transpose(0, 2, 1).astype(ml_dtypes.float8_e4m3fn))
    at = np.ascontiguousarray(audio.transpose(0, 2, 1).astype(ml_dtypes.bfloat16))

    in_maps = []
    for i in range(N_CORES):
        in_maps.append(
            {
                "xt": np.ascontiguousarray(xt[i * BL : (i + 1) * BL]),
                "at": np.ascontiguousarray(at[i * BL : (i + 1) * BL]),
                "w2t": w2t,
                "u": u,
                "Wv": wv,
                "bv": bv,
            }
        )
    return in_maps


def kernel(**inputs):
    nc = _get_nc()
    in_maps = make_in_maps(inputs)
    res = run_bass_kernel_spmd(nc, in_maps, core_ids=list(range(N_CORES)))
    return np.concatenate(
        [res.results[i]["out"].astype(np.float32) for i in range(N_CORES)], axis=0
    )


# revision 22
# speedup vs baseline: 2.0761x; 1.0128x over previous
"""Cross-attention block kernel for Trainium2 (8 NeuronCores, data-parallel).

Reference computation, for full inputs:
    Q = x @ Wq + bq            [B, HW, D]
    K = a @ Wk + bk            [B, S, D]
    V = a @ Wv + bv            [B, S, D]
    out = softmax(Q K^T / sqrt(D)) @ V

Algebraic restructuring (exact, exploits HW=4096 >> S=1024):
    Q K^T = x (Wq Wk^T) a^T + x (Wq bk) + (bq Wk^T) a^T + bq.bk
  The bk terms are constant across s, so they cancel in softmax. Host
  precomputes W2T = Wk Wq^T [da, dq] and u = Wk bq [da]; the device then
  computes G_b = W2T^T-contract-aT [dq, s] and r_b[s] = u . a_b[s], so
    logits = (x @ G_b + r_b) * scale
  replacing the [HW,512]x[512,512] Q projection AND the K projection with
  one [S,512]-sized build per batch.

Precision strategy (harness gate: rel err < 2e-2; measured 1.32e-2):
  - scores matmul in fp8 e4m3 with MatmulPerfMode.DoubleRow: 2 k-tiles
    (256 contraction) per pass at 0.5 cycles/row -> 4x fp32r throughput.
  - attn@V, V projection, G build in bf16 (full PE rate): fp8 there would
    put ~3% error directly on the output, over the gate.
  - x is transposed AND quantized to fp8 on the host (pure layout/dtype
    prep, like the batch sharding); audio transposed to bf16 d-major.
    Both are stored pre-blocked so every DMA is one contiguous run per
    partition. Output written bf16, blocked; host inverse-permutes.

Sharding: batch (B=16) split across 8 cores, 2 batches per core. Weights
replicated. No collectives.

Per-core schedule:
  per batch: DMA aT (halves); G8 = W2T^T @ aT (fp8 out, c-outer over the
    4 score PSUM banks so it starts as soon as the first w2t chunk and
    audio half land); rb[s] = scale*(u . a[s]) via N=1 matmuls;
    V = aT^T @ Wv + bv (bf16; bv arrives host-broadcast, no PE work).
  per 512-row block of x: scoresT = G8^T @ xT8 (DoubleRow, PSUM f32);
    ex = Exp(scale*scores + rb) on ACT (the only ACT work -> one table);
    out = ex^T @ V with an interleaved ones-column matmul accumulating
    the softmax denominator; DVE applies the reciprocal per-partition.
  The attn@V of block N is emitted interleaved into block N+1's score
  matmuls (software pipeline, depth 1): ACT Exp throughput (~600ns per
  s-chunk) would otherwise stall the in-order PE queue, since PE's own
  score work per chunk is only ~215ns. With 4 score PSUM banks the PE
  never waits on ACT. The final flush drains per-d-chunk (and the last
  chunk in quarters) so the tail DMA overlaps the last matmuls.
"""

from contextlib import ExitStack

import numpy as np
import ml_dtypes

import concourse.bass as bass
import concourse.bacc as bacc
import concourse.mybir as mybir
import concourse.tile as tile
from concourse.bass_utils import run_bass_kernel_spmd

P = 128
D = 512          # d_query == d_audio == d_out
CD = D // P      # 4 chunks of the feature dim
HW = 4096        # queries per batch
S = 1024         # keys per batch
SC = S // P      # 8 s-chunks
HWB = 512        # hw rows processed per block
NBLK = HW // HWB
B_FULL = 16
N_CORES = 8
BL = B_FULL // N_CORES  # 2 batches per core
SCALE = 1.0 / float(np.sqrt(D))
WPACK = CD * D + CD + D  # wv (CD*D) + u (CD) + bv broadcast (D), bf16 cols

f32 = mybir.dt.float32
f32r = mybir.dt.float32r
bf16 = mybir.dt.bfloat16
fp8 = mybir.dt.float8e4
AFT = mybir.ActivationFunctionType
DR = mybir.MatmulPerfMode.DoubleRow


def build_nc():
    nc = bacc.Bacc("TRN2", target_bir_lowering=False, debug=False)

    # Host-prepared layouts (one contiguous run per partition per DMA):
    #   xt[b, blk, p, c, n] = fp8(x[b, blk*512+n, c*128+p])
    #   at[b, t, p, c, n]   = bf16(audio[b, t*512+n, c*128+p])
    #   w2t[p, c, n]        = bf16((Wk Wq^T)[c*128+p, n])
    #   wpack[p, :]         = [Wv rows | u | bv broadcast], bf16
    #   out[b, blk, p, h, n] -> host permutes back to [b, hw, d]
    xt = nc.dram_tensor("xt", [BL, NBLK, P, CD, HWB], fp8, kind="ExternalInput").ap()
    at = nc.dram_tensor("at", [BL, 2, P, CD, HWB], bf16, kind="ExternalInput").ap()
    w2t = nc.dram_tensor("w2t", [P, CD, HWB], bf16, kind="ExternalInput").ap()
    wpack = nc.dram_tensor("wpack", [P, WPACK], bf16, kind="ExternalInput").ap()
    out = nc.dram_tensor("out", [BL, NBLK, P, CD, HWB], bf16, kind="ExternalOutput").ap()

    with tile.TileContext(nc) as tc:
        with ExitStack() as ctx:
            _body(ctx, tc, xt, at, w2t, wpack, out)

    nc.compile()
    return nc


def _body(ctx, tc, xt, at, w2t, wpack, out):
    nc = tc.nc

    const_pool = ctx.enter_context(tc.tile_pool(name="const", bufs=1))
    batch_pool = ctx.enter_context(tc.tile_pool(name="batch", bufs=1))
    # aT double-buffered in its own pool: the next batch's audio DMA must
    # carry no WAR wait, or it head-of-line-blocks every later DMA on the
    # in-order SP queue.
    a_pool = ctx.enter_context(tc.tile_pool(name="apool", bufs=2))
    work_pool = ctx.enter_context(tc.tile_pool(name="work", bufs=2))
    small_pool = ctx.enter_context(tc.tile_pool(name="small", bufs=4))
    psum_mm = ctx.enter_context(tc.tile_pool(name="pmm", bufs=2, space="PSUM"))
    psum_sc = ctx.enter_context(tc.tile_pool(name="psc", bufs=4, space="PSUM"))
    psum_den = ctx.enter_context(tc.tile_pool(name="pden", bufs=2, space="PSUM"))

    ones_f = const_pool.tile([P, 2], f32)
    nc.gpsimd.memset(ones_f, 1.0)
    ones_col = const_pool.tile([P, 2], bf16)
    nc.vector.tensor_copy(ones_col, ones_f)

    # critical-path-first DMA order: the first G matmuls need w2t chunk 0
    # and audio half 0; everything else trails.
    w2t_sb = const_pool.tile([P, CD, HWB], bf16)
    nc.sync.dma_start(w2t_sb[:, 0:1, :], w2t[:, 0:1, :])
    aT0 = a_pool.tile([P, CD, S], bf16, tag="aT", name="aT0")
    nc.sync.dma_start(aT0[:, :, 0:HWB], at[0, 0])
    nc.sync.dma_start(w2t_sb[:, 1:CD, :], w2t[:, 1:CD, :])
    wp_sb = const_pool.tile([P, WPACK], bf16)
    nc.sync.dma_start(wp_sb, wpack)
    nc.sync.dma_start(aT0[:, :, HWB:S], at[0, 1])
    xt_pre0 = work_pool.tile([P, CD, HWB], fp8, tag="xT", name="xt_pre0")
    nc.sync.dma_start(xt_pre0, xt[0, 0])

    wv_sb = wp_sb[:, 0 : CD * HWB].rearrange("p (c n) -> p c n", c=CD)
    u_col = wp_sb[:, CD * HWB : CD * HWB + CD]
    bv_bc = wp_sb[:, CD * HWB + CD : WPACK]

    xt_pre = {(0, 0): xt_pre0}
    a_tiles = {0: aT0}
    for b in range(BL):
        # --- per-batch: aT DMA, G8, rb, V ------------------------------
        aT = a_tiles.pop(b)
        if b + 1 < BL:
            # prefetch next batch's audio now: fresh buffer, no WAR wait
            aT_next = a_pool.tile([P, CD, S], bf16, tag="aT")
            for t in range(2):
                nc.sync.dma_start(aT_next[:, :, t * HWB : (t + 1) * HWB], at[b + 1, t])
            a_tiles[b + 1] = aT_next
        g8 = batch_pool.tile([P, CD, S], fp8, tag="g8")
        v_sb = batch_pool.tile([P, SC, D], bf16, tag="v")
        rb = batch_pool.tile([P, SC], f32, tag="rb")

        for half in range(2):
            hs = slice(half * HWB, (half + 1) * HWB)
            # G8[dq, s] = sum_da W2T[da, dq] * aT[da, s], fp8 out.
            # c-outer over the 4 score-psum banks: the first four matmuls
            # need only w2t chunk 0, which is the first DMA of the kernel.
            g_ps = []
            for m in range(CD):
                t = psum_sc.tile([P, HWB], f32, tag="sc", name=f"g_ps{m}")
                g_ps.append(t)
            for c in range(CD):
                for m in range(CD):
                    nc.tensor.matmul(
                        g_ps[m],
                        w2t_sb[:, c, m * P : (m + 1) * P],
                        aT[:, c, hs],
                        start=(c == 0),
                        stop=(c == CD - 1),
                    )
            for m in range(CD):
                nc.vector.tensor_copy(g8[:, m, hs], g_ps[m])

            # V[s, do] = sum_da aT[da, s] * Wv[da, do] + bv
            for g in range(half * 4, half * 4 + 4):
                mm_ps = psum_mm.tile([P, D], f32, tag="mm")
                for c in range(CD):
                    nc.tensor.matmul(
                        mm_ps,
                        aT[:, c, g * P : (g + 1) * P],
                        wv_sb[:, c, :],
                        start=(c == 0),
                        stop=(c == CD - 1),
                    )
                nc.vector.tensor_add(v_sb[:, g, :], mm_ps, bv_bc)

            # rb[s] = SCALE * sum_da u[da] * aT[da, s], per-partition layout
            for g in range(half * 4, half * 4 + 4):
                rb_ps = psum_den.tile([P, 2], f32, tag="den")
                for c in range(CD):
                    nc.tensor.matmul(
                        rb_ps[:, 0:1],
                        aT[:, c, g * P : (g + 1) * P],
                        u_col[:, c : c + 1],
                        start=(c == 0),
                        stop=(c == CD - 1),
                    )
                nc.vector.tensor_scalar_mul(rb[:, g : g + 1], rb_ps[:, 0:1], SCALE)

        # --- hw blocks, software-pipelined: attn@V of block N runs
        # interleaved with the score matmuls of block N+1 ----------------
        def emit_scores(g, xT8, ex):
            sc_ps = psum_sc.tile([P, HWB], f32, tag="sc")
            for nh in range(2):
                for kp in range(2):
                    nc.tensor.matmul(
                        sc_ps[:, nh * 256 : (nh + 1) * 256],
                        g8[:, 2 * kp : 2 * kp + 2, g * P : (g + 1) * P],
                        xT8[:, 2 * kp : 2 * kp + 2, nh * 256 : (nh + 1) * 256],
                        start=(kp == 0),
                        stop=(kp == 1),
                        perf_mode=DR,
                    )
            nc.scalar.activation(
                ex[:, g, :], sc_ps, AFT.Exp, bias=rb[:, g : g + 1], scale=SCALE
            )

        def emit_av(h, ex, out_sb):
            o_ps = psum_mm.tile([P, D], f32, tag="mm")
            d_ps = psum_den.tile([P, 2], f32, tag="den")
            for g in range(SC):
                lhs = ex[:, g, h * P : (h + 1) * P]
                nc.tensor.matmul(
                    o_ps, lhs, v_sb[:, g, :], start=(g == 0), stop=(g == SC - 1)
                )
                nc.tensor.matmul(
                    d_ps, lhs, ones_col, start=(g == 0), stop=(g == SC - 1)
                )
            rec = small_pool.tile([P, 1], f32, tag="rec")
            nc.vector.reciprocal(rec, d_ps[:, 0:1])
            nc.vector.tensor_scalar_mul(out_sb[:, h, :], o_ps, rec)
            return o_ps, rec

        pend = None
        for blk in range(NBLK):
            xT8 = xt_pre.pop((b, blk), None)
            if xT8 is None:
                xT8 = work_pool.tile([P, CD, HWB], fp8, tag="xT")
                nc.sync.dma_start(xT8, xt[b, blk])
            ex = work_pool.tile([P, SC, HWB], bf16, tag="ex")
            if pend is not None:
                p_blk, p_ex, p_out = pend
            for g in range(4):
                emit_scores(g, xT8, ex)
            if pend is not None:
                emit_av(0, p_ex, p_out)
            for g in range(4, 6):
                emit_scores(g, xT8, ex)
            if pend is not None:
                emit_av(1, p_ex, p_out)
            for g in range(6, 8):
                emit_scores(g, xT8, ex)
            if pend is not None:
                emit_av(2, p_ex, p_out)
                emit_av(3, p_ex, p_out)
                nc.sync.dma_start(out[b, p_blk], p_out)
            out_sb = work_pool.tile([P, CD, D], bf16, tag="o")
            pend = (blk, ex, out_sb)

        # flush: drain the last block per d-chunk so the tail DVE scale +
        # DMA overlap the remaining matmuls (finer splits lose: each DMA
        # carries ~1.3us of fixed HWDGE/DGE latency)
        p_blk, p_ex, p_out = pend
        for h in range(CD):
            emit_av(h, p_ex, p_out)
            nc.sync.dma_start(out[b, p_blk][:, h, :], p_out[:, h, :])


_NC_CACHE = None


def _get_nc():
    global _NC_CACHE
    if _NC_CACHE is None:
        _NC_CACHE = build_nc()
    return _NC_CACHE


def make_in_maps(inputs):
    x = np.asarray(inputs["x"], dtype=np.float32)
    audio = np.asarray(inputs["audio_embed"], dtype=np.float32)
    wq = np.asarray(inputs["Wq"], dtype=np.float32)
    bq = np.asarray(inputs["bq"], dtype=np.float32)
    wk = np.asarray(inputs["Wk"], dtype=np.float32)
    wv = np.asarray(inputs["Wv"], dtype=np.float32)
    bv = np.asarray(inputs["bv"], dtype=np.float32)

    # Host-side weight folding (exact, standard inference practice):
    #   W2T[da, dq] = sum_do Wk[da, do] Wq[dq, do]   and   u = Wk @ bq.
    # bk drops out entirely: its logit contribution is constant over s.
    w2t = (wk @ wq.T).astype(ml_dtypes.bfloat16)
    w2t = np.ascontiguousarray(w2t.reshape(CD, P, HWB).transpose(1, 0, 2))
    u = (wk @ bq).astype(ml_dtypes.bfloat16)

    # wpack rows: [Wv (row da=c*128+p) | u | bv broadcast]
    wv_b = wv.astype(ml_dtypes.bfloat16).reshape(CD, P, D).transpose(1, 0, 2)
    wpack = np.concatenate(
        [
            wv_b.reshape(P, CD * D),
            u.reshape(CD, P).T,
            np.broadcast_to(bv.astype(ml_dtypes.bfloat16), (P, D)),
        ],
        axis=1,
    )
    wpack = np.ascontiguousarray(wpack)

    # Layout/dtype prep: d-major, block-contiguous; x straight to fp8.
    xt = x.astype(ml_dtypes.float8_e4m3fn)
    xt = np.ascontiguousarray(
        xt.reshape(B_FULL, NBLK, HWB, CD, P).transpose(0, 1, 4, 3, 2)
    )
    at = audio.astype(ml_dtypes.bfloat16)
    at = np.ascontiguousarray(
        at.reshape(B_FULL, 2, HWB, CD, P).transpose(0, 1, 4, 3, 2)
    )

    in_maps = []
    for i in range(N_CORES):
        in_maps.append(
            {
                "xt": np.ascontiguousarray(xt[i * BL : (i + 1) * BL]),
                "at": np.ascontiguousarray(at[i * BL : (i + 1) * BL]),
                "w2t": w2t,
                "wpack": wpack,
            }
        )
    return in_maps


def kernel(**inputs):
    nc = _get_nc()
    in_maps = make_in_maps(inputs)
    res = run_bass_kernel_spmd(nc, in_maps, core_ids=list(range(N_CORES)))
    # out[b, blk, p, h, n] -> [b, blk*512 + h*128 + p, n]
    return np.ascontiguousarray(
        np.concatenate(
            [res.results[i]["out"].astype(np.float32) for i in range(N_CORES)], axis=0
        )
        .transpose(0, 1, 3, 2, 4)
        .reshape(B_FULL, HW, D)
    )
